# revision 2
# baseline (speedup 1.0000x reference)
"""Trainium2 Bass kernel for BaseRelationNetwork forward pass (v2, bf16 path).

Reference computation (per batch row b):
    pairs (i<j) of C=16 channels, P=120 pairs
    h1 = relu(concat(x_i, x_j) @ W1 + b1)      # W1 [2F, H]
    h2 = relu(h1 @ W2 + b2)
    out = mean_p(h2 @ W3 + b3)                 # [B, H]

Structure:
  1. W1 = [W1a; W1b]; ya = x @ W1a, yb = x @ W1b computed once per channel.
     h1[p=(i,j)] = relu(ya_i + yb_j + b1)  (DVE pair-add + relu).
  2. mean over pairs commutes with layer 3: out = (mean_p h2) @ W3 + b3.

Implementation notes:
  - All on-chip data is bf16 (inputs cast on host): halves DMA bytes, enables
    DVE 2x (tensor_tensor) / 4x (tensor_scalar) modes. rel err ~2e-3 vs the
    2e-2 gate.
  - 4-chunk software pipeline; PE issue order A0 A1 A2 C0 A3 F0 C1 F1 C2 F2
    C3 F3 E keeps the tensor engine fed.
  - Pair-sum: PE identity-matmul accumulates the 4 su-blocks of h2 into one
    PSUM tile per (chunk, m); DVE reduces the remaining 30 pairs.
  - start=True on a matmul clears the WHOLE psum bank: only the first matmul
    touching a bank may set it when several accumulation regions share one.
  - The timing loop body holds `unroll` copies of the kernel; all big tiles
    are shared (region-level WAR tracking lets body k+1's head DMAs overlap
    body k's tail compute), so the For_i all-engine reset barrier is paid
    once per `unroll` kernels.
  - "hd" head tensor = [W1 ktiles a0..b1 | x chunk0 k0-3 | W1 a4..b7 | x
    chunk0 k4-7]: two 512 KB DMAs so stage A can start ~3.5us after the
    barrier; a few warm-up matmuls on resident data keep the PE HAM
    un-throttled across the iteration boundary.

Sharding: data-parallel over batch. 512 rows / 8 cores = 64 rows per core,
4 chunks of 16 rows; weights replicated.
"""

import contextlib
import sys

if "/opt/trn_rl_repo" not in sys.path:
    sys.path.insert(0, "/opt/trn_rl_repo")

import numpy as np
import ml_dtypes

import concourse.bass as bass
import concourse.mybir as mybir
import concourse.tile as tile
from concourse import bacc
from concourse.bass_utils import run_bass_kernel_spmd

B, C, F, H = 512, 16, 1024, 256
N_CORES = 8
BL = B // N_CORES          # 64 local batch rows per core
NH = 4                     # chunks per core
BH = BL // NH              # 16 rows per chunk
CTOK = BH * C              # 256 tokens per chunk
TOK = BL * C               # 1024 tokens per core
P = C * (C - 1) // 2       # 120 pairs
PPG = 30                   # pairs per stage-C group
GW = PPG * BH              # 480 cols per group
NSU = P // PPG             # 4 groups (su blocks) per chunk
PCH = P * BH               # 1920 pair-cols per chunk
KT1 = F // 128             # 8 k-tiles for layer-1 contraction

F32 = mybir.dt.float32
BF16 = mybir.dt.bfloat16

PAIR_BASE = [0] * C
for _i in range(1, C):
    PAIR_BASE[_i] = PAIR_BASE[_i - 1] + (C - _i)

AF = mybir.ActivationFunctionType
ALU = mybir.AluOpType


def build_module(loop_iters: int = 1, unroll: int = 1):
    nc = bacc.Bacc("TRN2", target_bir_lowering=False, debug=True)

    # hd: [w1 kk0-3 | x c0 k0-3 | w1 kk4-7 | x c0 k4-7], each block 1024 cols
    hd_d = nc.dram_tensor("hd", [128, 4096], BF16, kind="ExternalInput")
    xt_d = nc.dram_tensor("xt", [F, TOK], BF16, kind="ExternalInput")
    w1_d = nc.dram_tensor("w1", [2 * F, H], BF16, kind="ExternalInput")
    w2_d = nc.dram_tensor("w2", [H, H], BF16, kind="ExternalInput")
    w3_d = nc.dram_tensor("w3", [H, H], BF16, kind="ExternalInput")
    bp_d = nc.dram_tensor("bias_pack", [128, 6], F32, kind="ExternalInput")
    id_d = nc.dram_tensor("ident", [128, 128], BF16, kind="ExternalInput")
    out_d = nc.dram_tensor("outT", [H, BL], F32, kind="ExternalOutput")

    with tile.TileContext(nc) as tc:
        with (
            tc.tile_pool(name="xpool", bufs=1) as xpool,
            tc.tile_pool(name="wpool", bufs=1) as wpool,
            tc.tile_pool(name="ypool", bufs=1) as ypool,
            tc.tile_pool(name="hpool", bufs=1) as hpool,
            tc.tile_pool(name="spool", bufs=1) as spool,
            tc.tile_pool(name="psA", bufs=2, space="PSUM") as psA_pool,
            tc.tile_pool(name="psC", bufs=2, space="PSUM") as psC_pool,
        ):
            loop_cm = (
                tc.For_i(0, loop_iters, 1)
                if loop_iters > 1
                else contextlib.nullcontext()
            )
            with loop_cm:
                hd = xpool.tile([128, 4096], BF16, tag="hd", name="hd")
                # xts holds chunks 1..3 only (chunk 0 lives in hd)
                xts = xpool.tile([128, KT1, 3 * CTOK], BF16, tag="xts", name="xts")
                # w1big holds interleaved ktiles kk8-15 (kk0-7 live in hd)
                w1big = wpool.tile([128, KT1, H], BF16, tag="w1big", name="w1big")
                w2t = wpool.tile([128, 2, H], BF16, tag="w2t", name="w2t")
                w3t = wpool.tile([128, 2, H], BF16, tag="w3t", name="w3t")
                bp = wpool.tile([128, 6], F32, tag="bp", name="bp")
                idt = wpool.tile([128, 128], BF16, tag="idt", name="idt")
                # y_all free layout: [m(4), ch(NH), c(C), b(BH)]
                y_all = ypool.tile([128, 4, TOK], BF16, tag="y_all", name="y_all")
                # h1 free layout: [t(2), ch(NH), p(P), b(BH)]
                h1all = hpool.tile([128, 2, NH * PCH], BF16, tag="h1all", name="h1all")
                # h2 free layout per m: [ch(NH), su(NSU), pp(PPG), b(BH)]
                h2sb = [
                    spool.tile([128, NH * PCH], BF16, tag=f"h2_{m}", name=f"h2_{m}")
                    for m in range(2)
                ]
                m2 = spool.tile([128, 2, BL], BF16, tag="m2", name="m2")
                ttmp = spool.tile([128, 2, GW], BF16, tag="ttmp", name="ttmp")
                osb = spool.tile([128, 2, BL], F32, tag="osb", name="osb")

                def bias(nm, t):
                    idx = {"b1": 0, "b2": 2, "b3": 4}[nm] + t
                    return bp[:, idx : idx + 1]

                w1v = w1_d.rearrange("(k p) h -> p k h", p=128)
                xtv = xt_d.rearrange("(k p) t -> p k t", p=128)

                def cs(ch):
                    return slice(ch * CTOK, (ch + 1) * CTOK)

                def w1sl(kk, ht):
                    """lhsT [128, 128] for interleaved ktile kk, out half ht."""
                    if kk < 4:
                        base = kk * 256 + ht * 128
                        return hd[:, base : base + 128]
                    if kk < 8:
                        base = 2048 + (kk - 4) * 256 + ht * 128
                        return hd[:, base : base + 128]
                    return w1big[:, kk - 8, ht * 128 : (ht + 1) * 128]

                def xsl(k, ch):
                    """moving operand [128, CTOK] for layer-1 ktile k, chunk ch."""
                    if ch == 0:
                        base = 1024 + (k // 4) * 2048 + (k % 4) * 256
                        return hd[:, base : base + CTOK]
                    return xts[:, k, (ch - 1) * CTOK : ch * CTOK]

                y5 = y_all.rearrange("p m (hh c b) -> p m hh c b", hh=NH, b=BH)
                h5 = h1all.rearrange("p t (hh pp b) -> p t hh pp b", hh=NH, b=BH)
                h2v = [
                    t.rearrange("p (hh su g) -> p hh su g", hh=NH, g=GW) for t in h2sb
                ]

                def emit_body(rep):
                    tag = f"r{rep}"

                    # ---- DMAs: head first on gpsimd; consts + output on sync
                    nc.gpsimd.dma_start(out=hd[:, :2048], in_=hd_d[:, :2048])
                    nc.gpsimd.dma_start(out=hd[:, 2048:], in_=hd_d[:, 2048:])
                    if rep == 0:
                        nc.sync.dma_start(out=bp[:], in_=bp_d[:])
                        nc.sync.dma_start(out=idt[:], in_=id_d[:])
                    nc.gpsimd.dma_start(out=w1big[:], in_=w1v[:, 8:16, :])
                    nc.gpsimd.dma_start(
                        out=xts[:, :, 0:CTOK], in_=xtv[:, :, cs(1)]
                    )
                    nc.gpsimd.dma_start(
                        out=w2t[:], in_=w2_d.rearrange("(k p) h -> p k h", p=128)
                    )
                    nc.gpsimd.dma_start(
                        out=xts[:, :, CTOK : 2 * CTOK], in_=xtv[:, :, cs(2)]
                    )
                    nc.gpsimd.dma_start(
                        out=xts[:, :, 2 * CTOK : 3 * CTOK], in_=xtv[:, :, cs(3)]
                    )
                    nc.sync.dma_start(
                        out=w3t[:], in_=w3_d.rearrange("(k p) h -> p k h", p=128)
                    )

                    # ---- PE warm keeper: a few matmuls on resident data so
                    # the HAM window stays busy across the loop boundary ----
                    psw = psA_pool.tile([128, 4, CTOK], F32, tag="psA",
                                        name=f"psW_{tag}")
                    nwarm = 10 if rep == 0 else 2
                    for wi in range(nwarm):
                        nc.tensor.matmul(
                            psw[:, 0, :CTOK],
                            idt[:],
                            idt[:, 0:1].broadcast_to([128, CTOK]),
                            start=(wi == 0), stop=(wi == nwarm - 1),
                        )

                    psa = {}

                    def stage_A(ch):
                        psa[ch] = psA_pool.tile(
                            [128, 4, CTOK], F32, tag="psA", name=f"psA_{ch}_{tag}"
                        )
                        for k in range(KT1):
                            for m in (0, 2, 1, 3):
                                w_half, ht = divmod(m, 2)
                                # start=True clears the WHOLE psum bank: only
                                # the first matmul touching each bank (m=0 ->
                                # bank0, m=2 -> bank1) may set it.
                                nc.tensor.matmul(
                                    psa[ch][:, m, :],
                                    w1sl(2 * k + w_half, ht),
                                    xsl(k, ch),
                                    start=(k == 0 and m in (0, 2)),
                                    stop=(k == KT1 - 1),
                                )

                    def stage_cp(ch):
                        p = psa[ch]
                        nc.scalar.activation(
                            y_all[:, 0, cs(ch)], p[:, 0, :], AF.Identity,
                            bias=bias("b1", 0),
                        )
                        nc.scalar.activation(
                            y_all[:, 1, cs(ch)], p[:, 1, :], AF.Identity,
                            bias=bias("b1", 1),
                        )
                        nc.scalar.copy(y_all[:, 2:4, cs(ch)], p[:, 2:4, :])

                    def stage_add(ch):
                        for i in range(C - 1):
                            nj = C - 1 - i
                            p0 = PAIR_BASE[i]
                            in0 = y5[:, 0:2, ch, i : i + 1, :].broadcast_to(
                                [128, 2, nj, BH]
                            )
                            in1 = y5[:, 2:4, ch, i + 1 :, :]
                            nc.vector.tensor_add(
                                h5[:, :, ch, p0 : p0 + nj, :], in0, in1
                            )

                    def stage_rl1(ch):
                        sl = h1all[:, :, ch * PCH : (ch + 1) * PCH]
                        nc.vector.tensor_scalar_max(sl, sl, 0.0)

                    def stage_C(ch):
                        for sp in range(2):
                            for m in range(2):
                                ps = psC_pool.tile(
                                    [128, 1024], F32, tag="psC",
                                    name=f"psC_{ch}_{sp}_{m}_{tag}",
                                )
                                for sub in range(2):
                                    s = sp * 2 + sub
                                    for k in range(2):
                                        nc.tensor.matmul(
                                            ps[:, sub * 512 : sub * 512 + GW],
                                            w2t[:, k, m * 128 : (m + 1) * 128],
                                            h1all[
                                                :,
                                                k,
                                                ch * PCH + s * GW : ch * PCH
                                                + (s + 1) * GW,
                                            ],
                                            start=(k == 0),
                                            stop=(k == 1),
                                        )
                                psv = ps.rearrange("p (u g) -> p u g", g=512)[
                                    :, :, :GW
                                ]
                                nc.scalar.activation(
                                    h2v[m][:, ch, sp * 2 : (sp + 1) * 2, :],
                                    psv,
                                    AF.Relu,
                                    bias=bias("b2", m),
                                )

                    psr = {}

                    def stage_F(ch):
                        # m=0 su-sum on PE (identity matmul); m=1 goes via a
                        # DVE tensor-tensor tree in stage_red (2x bf16) to
                        # offload the power-limited tensor engine.
                        pr = psC_pool.tile(
                            [128, 1024], F32, tag="psC", name=f"psR_{ch}_0_{tag}"
                        )
                        for su in range(NSU):
                            nc.tensor.matmul(
                                pr[:, :GW],
                                idt[:],
                                h2v[0][:, ch, su, :],
                                start=(su == 0),
                                stop=(su == NSU - 1),
                            )
                        psr[(ch, 0)] = pr

                    def stage_red(ch):
                        pr = psr.pop((ch, 0))
                        v = (
                            pr[:, :GW]
                            .rearrange("q (pp b) -> q pp b", b=BH)
                            .transpose([0, 2, 1])
                        )
                        with nc.allow_low_precision(reason="pair-mean to bf16"):
                            nc.vector.tensor_reduce(
                                m2[:, 0, ch * BH : (ch + 1) * BH],
                                v,
                                mybir.AxisListType.X,
                                ALU.add,
                            )
                        # m=1: su-tree on DVE (bf16 2x), then 30-pair reduce
                        nc.vector.tensor_add(
                            ttmp[:],
                            h2v[1][:, ch, 0:2, :],
                            h2v[1][:, ch, 2:4, :],
                        )
                        nc.vector.tensor_add(ttmp[:, 0, :], ttmp[:, 0, :], ttmp[:, 1, :])
                        v1 = (
                            ttmp[:, 0, :]
                            .rearrange("q (pp b) -> q pp b", b=BH)
                            .transpose([0, 2, 1])
                        )
                        with nc.allow_low_precision(reason="pair-mean to bf16"):
                            nc.vector.tensor_reduce(
                                m2[:, 1, ch * BH : (ch + 1) * BH],
                                v1,
                                mybir.AxisListType.X,
                                ALU.add,
                            )

                    # ---- pipelined emission ----
                    stage_A(0); stage_cp(0)
                    stage_A(1); stage_cp(1)
                    stage_add(0); stage_rl1(0)
                    stage_A(2); stage_cp(2)
                    stage_add(1); stage_rl1(1)
                    stage_C(0)
                    stage_A(3); stage_cp(3)
                    stage_add(2); stage_rl1(2)
                    stage_F(0); stage_red(0)
                    stage_C(1)
                    stage_add(3); stage_rl1(3)
                    stage_F(1); stage_red(1)
                    stage_C(2)
                    stage_F(2); stage_red(2)
                    stage_C(3)
                    stage_F(3); stage_red(3)

                    if rep == unroll - 1 and loop_iters > 1:
                        # tail warm burst: keep the PE HAM window busy through
                        # the red3/E tail and into the loop barrier
                        psw2 = psA_pool.tile([128, 4, CTOK], F32, tag="psA",
                                             name=f"psW2_{tag}")
                        for wi in range(8):
                            nc.tensor.matmul(
                                psw2[:, 0, :CTOK],
                                idt[:],
                                idt[:, 0:1].broadcast_to([128, CTOK]),
                                start=(wi == 0), stop=(wi == 7),
                            )

                    # ---- stage E: outT = m2 @ (W3/P) + b3 ----
                    psE = psA_pool.tile([128, 4, CTOK], F32, tag="psA",
                                        name=f"psE_{tag}")
                    for mo in range(2):
                        # mo*2: separate psum banks for the two groups
                        for k in range(2):
                            nc.tensor.matmul(
                                psE[:, mo * 2, :BL],
                                w3t[:, k, mo * 128 : (mo + 1) * 128],
                                m2[:, k, :],
                                start=(k == 0),
                                stop=(k == 1),
                            )
                        nc.vector.tensor_scalar_add(
                            osb[:, mo, :], psE[:, mo * 2, :BL], bias("b3", mo)
                        )
                    nc.sync.dma_start(
                        out=out_d.rearrange("(m p) b -> p m b", p=128), in_=osb[:]
                    )

                for rep in range(unroll):
                    emit_body(rep)

    nc.compile()
    return nc


_NC_CACHE = None


def _get_module():
    global _NC_CACHE
    if _NC_CACHE is None:
        _NC_CACHE = build_module()
    return _NC_CACHE


def make_in_maps(x, W1, b1, W2, b2, W3, b3):
    bf = ml_dtypes.bfloat16
    W1 = np.asarray(W1, dtype=np.float32)
    # interleave W1a/W1b k-tiles: row-block order a0,b0,a1,b1,...
    w1i = np.ascontiguousarray(
        W1.reshape(2, KT1, 128, H).transpose(1, 0, 2, 3).reshape(2 * F, H)
    ).astype(bf)
    w2 = np.ascontiguousarray(np.asarray(W2, dtype=np.float32)).astype(bf)
    w3 = np.ascontiguousarray(
        np.asarray(W3, dtype=np.float32) / np.float32(P)
    ).astype(bf)
    b1 = np.asarray(b1, dtype=np.float32)
    b2 = np.asarray(b2, dtype=np.float32)
    b3 = np.asarray(b3, dtype=np.float32)
    bias_pack = np.ascontiguousarray(
        np.stack([b1[:128], b1[128:], b2[:128], b2[128:], b3[:128], b3[128:]], axis=1),
        dtype=np.float32,
    )
    ident = np.eye(128, dtype=np.float32).astype(bf)
    w1k = np.asarray(w1i).reshape(16, 128, H)
    in_maps = []
    x = np.asarray(x, dtype=np.float32)
    for i in range(N_CORES):
        xs = x[i * BL : (i + 1) * BL]  # [BL, C, F]
        chunks = [
            xs[ch * BH : (ch + 1) * BH].transpose(1, 0, 2).reshape(CTOK, F)
            for ch in range(NH)
        ]
        xT = np.ascontiguousarray(np.concatenate(chunks, axis=0).T).astype(bf)
        xTk = np.asarray(xT).reshape(KT1, 128, TOK)
        # hd = [w1 kk0-3 | x c0 k0-3 | w1 kk4-7 | x c0 k4-7] per partition
        hd = np.concatenate(
            [np.asarray(w1k[kk]) for kk in range(4)]
            + [np.asarray(xTk[k, :, :CTOK]) for k in range(4)]
            + [np.asarray(w1k[kk]) for kk in range(4, 8)]
            + [np.asarray(xTk[k, :, :CTOK]) for k in range(4, 8)],
            axis=1,
        )
        in_maps.append(
            {
                "hd": np.ascontiguousarray(hd),
                "xt": xT,
                "w1": w1i,
                "w2": w2,
                "w3": w3,
                "bias_pack": bias_pack,
                "ident": ident,
            }
        )
    return in_maps


def kernel(x, W1, b1, W2, b2, W3, b3):
    nc = _get_module()
    in_maps = make_in_maps(x, W1, b1, W2, b2, W3, b3)
    res = run_bass_kernel_spmd(nc, in_maps, list(range(N_CORES)))
    out = np.empty((B, H), dtype=np.float32)
    for i in range(N_CORES):
        out[i * BL : (i + 1) * BL] = res.results[i]["outT"].T
    return out


# revision 3
# speedup vs baseline: 1.2892x; 1.2892x over previous
"""Trainium2 Bass kernel for BaseRelationNetwork forward pass (v2, bf16 path).

Reference computation (per batch row b):
    pairs (i<j) of C=16 channels, P=120 pairs
    h1 = relu(concat(x_i, x_j) @ W1 + b1)      # W1 [2F, H]
    h2 = relu(h1 @ W2 + b2)
    out = mean_p(h2 @ W3 + b3)                 # [B, H]

Structure:
  1. W1 = [W1a; W1b]; ya = x @ W1a, yb = x @ W1b computed once per channel.
     h1[p=(i,j)] = relu(ya_i + yb_j + b1)  (DVE pair-add + relu).
  2. mean over pairs commutes with layer 3: out = (mean_p h2) @ W3 + b3.

Implementation notes:
  - All on-chip data is bf16 (inputs cast on host): halves DMA bytes, enables
    DVE 2x (tensor_tensor) / 4x (tensor_scalar) modes. rel err ~2e-3 vs the
    2e-2 gate.
  - 4-chunk software pipeline; PE issue order A0 A1 A2 C0 A3 F0 C1 F1 C2 F2
    C3 F3 E keeps the tensor engine fed.
  - Pair-sum: PE identity-matmul accumulates the 4 su-blocks of h2 into one
    PSUM tile per (chunk, m); DVE reduces the remaining 30 pairs.
  - start=True on a matmul clears the WHOLE psum bank: only the first matmul
    touching a bank may set it when several accumulation regions share one.
  - The timing loop body holds `unroll` copies of the kernel; all big tiles
    are shared (region-level WAR tracking lets body k+1's head DMAs overlap
    body k's tail compute), so the For_i all-engine reset barrier is paid
    once per `unroll` kernels.
  - "hd" head tensor = [W1 ktiles a0..b1 | x chunk0 k0-3 | W1 a4..b7 | x
    chunk0 k4-7]: two 512 KB DMAs so stage A can start ~3.5us after the
    barrier; a few warm-up matmuls on resident data keep the PE HAM
    un-throttled across the iteration boundary.

Sharding: data-parallel over batch. 512 rows / 8 cores = 64 rows per core,
4 chunks of 16 rows; weights replicated.
"""

import contextlib
import sys

if "/opt/trn_rl_repo" not in sys.path:
    sys.path.insert(0, "/opt/trn_rl_repo")

import numpy as np
import ml_dtypes

import concourse.bass as bass
import concourse.mybir as mybir
import concourse.tile as tile
from concourse import bacc
from concourse.bass_utils import run_bass_kernel_spmd

B, C, F, H = 512, 16, 1024, 256
N_CORES = 8
BL = B // N_CORES          # 64 local batch rows per core
NH = 4                     # chunks per core
BH = BL // NH              # 16 rows per chunk
CTOK = BH * C              # 256 tokens per chunk
TOK = BL * C               # 1024 tokens per core
P = C * (C - 1) // 2       # 120 pairs
PPG = 30                   # pairs per stage-C group
GW = PPG * BH              # 480 cols per group
NSU = P // PPG             # 4 groups (su blocks) per chunk
PCH = P * BH               # 1920 pair-cols per chunk
KT1 = F // 128             # 8 k-tiles for layer-1 contraction

F32 = mybir.dt.float32
BF16 = mybir.dt.bfloat16

PAIR_BASE = [0] * C
for _i in range(1, C):
    PAIR_BASE[_i] = PAIR_BASE[_i - 1] + (C - _i)

AF = mybir.ActivationFunctionType
ALU = mybir.AluOpType


def build_module(loop_iters: int = 1, unroll: int = 1):
    nc = bacc.Bacc("TRN2", target_bir_lowering=False, debug=True)

    # hd: [w1 kk0-3 | x c0 k0-3 | w1 kk4-7 | x c0 k4-7], each block 1024 cols
    hd_d = nc.dram_tensor("hd", [128, 4096], BF16, kind="ExternalInput")
    xt_d = nc.dram_tensor("xt", [F, TOK], BF16, kind="ExternalInput")
    w1_d = nc.dram_tensor("w1", [2 * F, H], BF16, kind="ExternalInput")
    w2_d = nc.dram_tensor("w2", [H, H], BF16, kind="ExternalInput")
    w3_d = nc.dram_tensor("w3", [H, H], BF16, kind="ExternalInput")
    bp_d = nc.dram_tensor("bias_pack", [128, 6], F32, kind="ExternalInput")
    id_d = nc.dram_tensor("ident", [128, 128], BF16, kind="ExternalInput")
    out_d = nc.dram_tensor("outT", [H, BL], F32, kind="ExternalOutput")

    with tile.TileContext(nc) as tc:
        with (
            tc.tile_pool(name="xpool", bufs=1) as xpool,
            tc.tile_pool(name="wpool", bufs=1) as wpool,
            tc.tile_pool(name="ypool", bufs=1) as ypool,
            tc.tile_pool(name="hpool", bufs=1) as hpool,
            tc.tile_pool(name="spool", bufs=1) as spool,
            tc.tile_pool(name="psA", bufs=2, space="PSUM") as psA_pool,
            tc.tile_pool(name="psC", bufs=2, space="PSUM") as psC_pool,
        ):
            loop_cm = (
                tc.For_i(0, loop_iters, 1)
                if loop_iters > 1
                else contextlib.nullcontext()
            )
            with loop_cm:
                hd = xpool.tile([128, 4096], BF16, tag="hd", name="hd")
                # xts holds chunks 1..3 only (chunk 0 lives in hd)
                xts = xpool.tile([128, KT1, 3 * CTOK], BF16, tag="xts", name="xts")
                # w1big holds interleaved ktiles kk8-15 (kk0-7 live in hd)
                w1big = wpool.tile([128, KT1, H], BF16, tag="w1big", name="w1big")
                w2t = wpool.tile([128, 2, H], BF16, tag="w2t", name="w2t")
                w3t = wpool.tile([128, 2, H], BF16, tag="w3t", name="w3t")
                bp = wpool.tile([128, 6], F32, tag="bp", name="bp")
                idt = wpool.tile([128, 128], BF16, tag="idt", name="idt")
                # y_all free layout: [m(4), ch(NH), c(C), b(BH)]
                y_all = ypool.tile([128, 4, TOK], BF16, tag="y_all", name="y_all")
                # h1 free layout: [t(2), ch(NH), p(P), b(BH)]
                h1all = hpool.tile([128, 2, NH * PCH], BF16, tag="h1all", name="h1all")
                # h2 free layout per m: [ch(NH), su(NSU), pp(PPG), b(BH)]
                h2sb = [
                    spool.tile([128, NH * PCH], BF16, tag=f"h2_{m}", name=f"h2_{m}")
                    for m in range(2)
                ]
                m2 = spool.tile([128, 2, BL], BF16, tag="m2", name="m2")
                ttmp = spool.tile([128, 2, GW], BF16, tag="ttmp", name="ttmp")
                osb = spool.tile([128, 2, BL], F32, tag="osb", name="osb")

                def bias(nm, t):
                    idx = {"b1": 0, "b2": 2, "b3": 4}[nm] + t
                    return bp[:, idx : idx + 1]

                w1v = w1_d.rearrange("(k p) h -> p k h", p=128)
                xtv = xt_d.rearrange("(k p) t -> p k t", p=128)

                def cs(ch):
                    return slice(ch * CTOK, (ch + 1) * CTOK)

                def w1sl(kk, ht):
                    """lhsT [128, 128] for interleaved ktile kk, out half ht."""
                    if kk < 4:
                        base = kk * 256 + ht * 128
                        return hd[:, base : base + 128]
                    if kk < 8:
                        base = 2048 + (kk - 4) * 256 + ht * 128
                        return hd[:, base : base + 128]
                    return w1big[:, kk - 8, ht * 128 : (ht + 1) * 128]

                def xsl(k, ch):
                    """moving operand [128, CTOK] for layer-1 ktile k, chunk ch."""
                    if ch == 0:
                        base = 1024 + (k // 4) * 2048 + (k % 4) * 256
                        return hd[:, base : base + CTOK]
                    return xts[:, k, (ch - 1) * CTOK : ch * CTOK]

                y5 = y_all.rearrange("p m (hh c b) -> p m hh c b", hh=NH, b=BH)
                h5 = h1all.rearrange("p t (hh pp b) -> p t hh pp b", hh=NH, b=BH)
                h2v = [
                    t.rearrange("p (hh su g) -> p hh su g", hh=NH, g=GW) for t in h2sb
                ]

                def emit_body(rep):
                    tag = f"r{rep}"

                    # ---- DMAs: head first on gpsimd; consts + output on sync
                    nc.gpsimd.dma_start(out=hd[:, :2048], in_=hd_d[:, :2048])
                    nc.gpsimd.dma_start(out=hd[:, 2048:], in_=hd_d[:, 2048:])
                    if rep == 0:
                        nc.sync.dma_start(out=bp[:], in_=bp_d[:])
                        nc.sync.dma_start(out=idt[:], in_=id_d[:])
                    nc.gpsimd.dma_start(out=w1big[:], in_=w1v[:, 8:16, :])
                    nc.gpsimd.dma_start(
                        out=xts[:, :, 0:CTOK], in_=xtv[:, :, cs(1)]
                    )
                    nc.gpsimd.dma_start(
                        out=w2t[:], in_=w2_d.rearrange("(k p) h -> p k h", p=128)
                    )
                    nc.gpsimd.dma_start(
                        out=xts[:, :, CTOK : 2 * CTOK], in_=xtv[:, :, cs(2)]
                    )
                    nc.gpsimd.dma_start(
                        out=xts[:, :, 2 * CTOK : 3 * CTOK], in_=xtv[:, :, cs(3)]
                    )
                    nc.sync.dma_start(
                        out=w3t[:], in_=w3_d.rearrange("(k p) h -> p k h", p=128)
                    )

                    # ---- PE warm keeper: a few matmuls on resident data so
                    # the HAM window stays busy across the loop boundary ----
                    if rep == 0:
                        psw = psA_pool.tile([128, 4, CTOK], F32, tag="psA",
                                            name=f"psW_{tag}")
                        for wi in range(10):
                            nc.tensor.matmul(
                                psw[:, 0, :CTOK],
                                idt[:],
                                idt[:, 0:1].broadcast_to([128, CTOK]),
                                start=(wi == 0), stop=(wi == 9),
                            )

                    psa = {}

                    def stage_A(ch):
                        psa[ch] = psA_pool.tile(
                            [128, 4, CTOK], F32, tag="psA", name=f"psA_{ch}_{tag}"
                        )
                        for k in range(KT1):
                            for m in (0, 2, 1, 3):
                                w_half, ht = divmod(m, 2)
                                # start=True clears the WHOLE psum bank: only
                                # the first matmul touching each bank (m=0 ->
                                # bank0, m=2 -> bank1) may set it.
                                nc.tensor.matmul(
                                    psa[ch][:, m, :],
                                    w1sl(2 * k + w_half, ht),
                                    xsl(k, ch),
                                    start=(k == 0 and m in (0, 2)),
                                    stop=(k == KT1 - 1),
                                )

                    def stage_cp(ch):
                        p = psa[ch]
                        nc.scalar.activation(
                            y_all[:, 0, cs(ch)], p[:, 0, :], AF.Identity,
                            bias=bias("b1", 0),
                        )
                        nc.scalar.activation(
                            y_all[:, 1, cs(ch)], p[:, 1, :], AF.Identity,
                            bias=bias("b1", 1),
                        )
                        nc.scalar.copy(y_all[:, 2:4, cs(ch)], p[:, 2:4, :])

                    def stage_add(ch):
                        for i in range(C - 1):
                            nj = C - 1 - i
                            p0 = PAIR_BASE[i]
                            in0 = y5[:, 0:2, ch, i : i + 1, :].broadcast_to(
                                [128, 2, nj, BH]
                            )
                            in1 = y5[:, 2:4, ch, i + 1 :, :]
                            nc.vector.tensor_add(
                                h5[:, :, ch, p0 : p0 + nj, :], in0, in1
                            )

                    def stage_rl1(ch):
                        sl = h1all[:, :, ch * PCH : (ch + 1) * PCH]
                        nc.vector.tensor_scalar_max(sl, sl, 0.0)

                    def stage_C(ch):
                        for sp in range(2):
                            for m in range(2):
                                ps = psC_pool.tile(
                                    [128, 1024], F32, tag="psC",
                                    name=f"psC_{ch}_{sp}_{m}_{tag}",
                                )
                                for sub in range(2):
                                    s = sp * 2 + sub
                                    for k in range(2):
                                        nc.tensor.matmul(
                                            ps[:, sub * 512 : sub * 512 + GW],
                                            w2t[:, k, m * 128 : (m + 1) * 128],
                                            h1all[
                                                :,
                                                k,
                                                ch * PCH + s * GW : ch * PCH
                                                + (s + 1) * GW,
                                            ],
                                            start=(k == 0),
                                            stop=(k == 1),
                                        )
                                psv = ps.rearrange("p (u g) -> p u g", g=512)[
                                    :, :, :GW
                                ]
                                nc.scalar.activation(
                                    h2v[m][:, ch, sp * 2 : (sp + 1) * 2, :],
                                    psv,
                                    AF.Relu,
                                    bias=bias("b2", m),
                                )

                    psr = {}

                    def stage_F(ch):
                        # m=0 su-sum on PE (identity matmul); m=1 goes via a
                        # DVE tensor-tensor tree in stage_red (2x bf16) to
                        # offload the power-limited tensor engine.
                        # psR lives in the psA pool: its slots recur a whole
                        # chunk apart, so the DVE-read WAR never stalls the
                        # stage-C psC rotation.
                        prt = psA_pool.tile(
                            [128, 4, CTOK], F32, tag="psA", name=f"psR_{ch}_0_{tag}"
                        )
                        pr = prt.rearrange("p a b -> p (a b)")
                        for su in range(NSU):
                            nc.tensor.matmul(
                                pr[:, :GW],
                                idt[:],
                                h2v[0][:, ch, su, :],
                                start=(su == 0),
                                stop=(su == NSU - 1),
                            )
                        psr[(ch, 0)] = pr

                    def stage_red(ch):
                        pr = psr.pop((ch, 0))
                        v = (
                            pr[:, :GW]
                            .rearrange("q (pp b) -> q pp b", b=BH)
                            .transpose([0, 2, 1])
                        )
                        with nc.allow_low_precision(reason="pair-mean to bf16"):
                            nc.vector.tensor_reduce(
                                m2[:, 0, ch * BH : (ch + 1) * BH],
                                v,
                                mybir.AxisListType.X,
                                ALU.add,
                            )
                        # m=1: su-tree on DVE (bf16 2x), then 30-pair reduce
                        nc.vector.tensor_add(
                            ttmp[:],
                            h2v[1][:, ch, 0:2, :],
                            h2v[1][:, ch, 2:4, :],
                        )
                        nc.vector.tensor_add(ttmp[:, 0, :], ttmp[:, 0, :], ttmp[:, 1, :])
                        v1 = (
                            ttmp[:, 0, :]
                            .rearrange("q (pp b) -> q pp b", b=BH)
                            .transpose([0, 2, 1])
                        )
                        with nc.allow_low_precision(reason="pair-mean to bf16"):
                            nc.vector.tensor_reduce(
                                m2[:, 1, ch * BH : (ch + 1) * BH],
                                v1,
                                mybir.AxisListType.X,
                                ALU.add,
                            )

                    # ---- pipelined emission ----
                    stage_A(0); stage_cp(0)
                    stage_A(1); stage_cp(1)
                    stage_add(0); stage_rl1(0)
                    stage_A(2); stage_cp(2)
                    stage_add(1); stage_rl1(1)
                    stage_C(0)
                    stage_A(3); stage_cp(3)
                    stage_add(2); stage_rl1(2)
                    stage_F(0); stage_red(0)
                    stage_C(1)
                    stage_add(3); stage_rl1(3)
                    stage_F(1); stage_red(1)
                    stage_C(2)
                    stage_F(2); stage_red(2)
                    stage_C(3)
                    stage_F(3); stage_red(3)

                    if rep == unroll - 1 and loop_iters > 1:
                        # tail warm burst: keep the PE HAM window busy through
                        # the red3/E tail and into the loop barrier
                        psw2 = psA_pool.tile([128, 4, CTOK], F32, tag="psA",
                                             name=f"psW2_{tag}")
                        for wi in range(8):
                            nc.tensor.matmul(
                                psw2[:, 0, :CTOK],
                                idt[:],
                                idt[:, 0:1].broadcast_to([128, CTOK]),
                                start=(wi == 0), stop=(wi == 7),
                            )

                    # ---- stage E: outT = m2 @ (W3/P) + b3 ----
                    psE = psA_pool.tile([128, 4, CTOK], F32, tag="psA",
                                        name=f"psE_{tag}")
                    for mo in range(2):
                        # mo*2: separate psum banks for the two groups
                        for k in range(2):
                            nc.tensor.matmul(
                                psE[:, mo * 2, :BL],
                                w3t[:, k, mo * 128 : (mo + 1) * 128],
                                m2[:, k, :],
                                start=(k == 0),
                                stop=(k == 1),
                            )
                        nc.vector.tensor_scalar_add(
                            osb[:, mo, :], psE[:, mo * 2, :BL], bias("b3", mo)
                        )
                    nc.sync.dma_start(
                        out=out_d.rearrange("(m p) b -> p m b", p=128), in_=osb[:]
                    )

                for rep in range(unroll):
                    emit_body(rep)

    nc.compile()
    return nc


_NC_CACHE = None


def _get_module():
    global _NC_CACHE
    if _NC_CACHE is None:
        _NC_CACHE = build_module()
    return _NC_CACHE


def make_in_maps(x, W1, b1, W2, b2, W3, b3):
    bf = ml_dtypes.bfloat16
    W1 = np.asarray(W1, dtype=np.float32)
    # interleave W1a/W1b k-tiles: row-block order a0,b0,a1,b1,...
    w1i = np.ascontiguousarray(
        W1.reshape(2, KT1, 128, H).transpose(1, 0, 2, 3).reshape(2 * F, H)
    ).astype(bf)
    w2 = np.ascontiguousarray(np.asarray(W2, dtype=np.float32)).astype(bf)
    w3 = np.ascontiguousarray(
        np.asarray(W3, dtype=np.float32) / np.float32(P)
    ).astype(bf)
    b1 = np.asarray(b1, dtype=np.float32)
    b2 = np.asarray(b2, dtype=np.float32)
    b3 = np.asarray(b3, dtype=np.float32)
    bias_pack = np.ascontiguousarray(
        np.stack([b1[:128], b1[128:], b2[:128], b2[128:], b3[:128], b3[128:]], axis=1),
        dtype=np.float32,
    )
    ident = np.eye(128, dtype=np.float32).astype(bf)
    w1k = np.asarray(w1i).reshape(16, 128, H)
    in_maps = []
    x = np.asarray(x, dtype=np.float32)
    for i in range(N_CORES):
        xs = x[i * BL : (i + 1) * BL]  # [BL, C, F]
        chunks = [
            xs[ch * BH : (ch + 1) * BH].transpose(1, 0, 2).reshape(CTOK, F)
            for ch in range(NH)
        ]
        xT = np.ascontiguousarray(np.concatenate(chunks, axis=0).T).astype(bf)
        xTk = np.asarray(xT).reshape(KT1, 128, TOK)
        # hd = [w1 kk0-3 | x c0 k0-3 | w1 kk4-7 | x c0 k4-7] per partition
        hd = np.concatenate(
            [np.asarray(w1k[kk]) for kk in range(4)]
            + [np.asarray(xTk[k, :, :CTOK]) for k in range(4)]
            + [np.asarray(w1k[kk]) for kk in range(4, 8)]
            + [np.asarray(xTk[k, :, :CTOK]) for k in range(4, 8)],
            axis=1,
        )
        in_maps.append(
            {
                "hd": np.ascontiguousarray(hd),
                "xt": xT,
                "w1": w1i,
                "w2": w2,
                "w3": w3,
                "bias_pack": bias_pack,
                "ident": ident,
            }
        )
    return in_maps


def kernel(x, W1, b1, W2, b2, W3, b3):
    nc = _get_module()
    in_maps = make_in_maps(x, W1, b1, W2, b2, W3, b3)
    res = run_bass_kernel_spmd(nc, in_maps, list(range(N_CORES)))
    out = np.empty((B, H), dtype=np.float32)
    for i in range(N_CORES):
        out[i * BL : (i + 1) * BL] = res.results[i]["outT"].T
    return out
